# revision 13
# baseline (speedup 1.0000x reference)
"""DnCNN-ablation (conv3x3->25ch, per-pixel 5x5 PixelConv, BN, ReLU) x14 layers.

Strategy: pure data parallel over batch B=8 -> 8 NeuronCores, one 3x256x256
image per core. The whole 14-layer net runs on-chip (SBUF resident).

Per-core layout ("pixel-partition"): a plane is stored as a tile
[128 partitions, SLOTS=2, PW=264] fp32 where partition p, slot s holds padded
image row r = p + 128*s; free position w maps image column w-2 (cols 0,1 and
258..263 are zero pads).

Per layer:
  1. conv k (25 planes) on the TensorEngine: contraction rows are 9
     h-shifted channel planes + a ones row (bias), grouped 5x along K
     (K=50/M=125 and K=30/M=75 pentads); the 3 w-shifts (dj) are
     PSUM-accumulated matmuls reading the same rhs at free offsets.
     rhs = [80, 8456] tile, row (g*10+q) = group-g row-span of the
     h-shifted channel plane q.  PSUM evacuated by DMA into
     pixel-partition k tiles.
  2. PixelConv y[c] = sum_p k_p * shift_p(x_c) as DVE tensor_tensor
     mult/add passes; h-shifts come from 5 physical h-shifted copies of
     each x plane (DMA-built), w-shifts are free-dim offsets.
  3. BN+ReLU in one ScalarE activation pass (per-partition scale/bias APs),
     writing the next layer's x.
"""

import numpy as np

L, C, H, W = 14, 3, 256, 256
OC = 25
PW = 264          # padded row width; w maps image col w-2
SLOTS = 2         # H / 128
G = 8             # conv row-groups
GR = H // G       # 32 rows per group
GF = GR * PW      # 8448 flat elems per group row-span
RHSF = 4 + GF + 4
NA = 5            # pentad A groups (0..4), K=50, M=125
NB = 3            # pentad B groups (5..7), K=30, M=75
EPS = 1e-5

_PROG_CACHE = {}


def _build_program(n_layers=L):
    import concourse.bacc as bacc
    import concourse.mybir as mybir
    from concourse.tile import TileContext

    f32 = mybir.dt.float32
    # Bacc (not raw Bass): its compile() runs generate_event_semaphores,
    # which splits multi-sem waits -- TRN2 instructions carry at most one
    # sync wait, and walrus rejects unsplit instructions.
    nc = bacc.Bacc("TRN2", target_bir_lowering=False)

    x_in = nc.dram_tensor("x", [C, H, W], f32, kind="ExternalInput")
    wa_in = nc.dram_tensor("wa", [3, n_layers, 10 * NA, 25 * NA], f32,
                           kind="ExternalInput")
    wb_in = nc.dram_tensor("wb", [3, n_layers, 10 * NB, 25 * NB], f32,
                           kind="ExternalInput")
    sc_in = nc.dram_tensor("bnscale", [128, n_layers * C], f32,
                           kind="ExternalInput")
    bi_in = nc.dram_tensor("bnbias", [128, n_layers * C], f32,
                           kind="ExternalInput")
    y_out = nc.dram_tensor("y", [C, H, W], f32, kind="ExternalOutput")

    with TileContext(nc) as tc:
        with (
            tc.tile_pool(name="main", bufs=1) as pool,
            tc.tile_pool(name="pa", bufs=2, space="PSUM") as pa_pool,
            tc.tile_pool(name="pb", bufs=2, space="PSUM") as pb_pool,
            tc.tile_pool(name="ks", bufs=3) as ks_pool,
        ):
            # persistent tiles
            X = [[pool.tile([128, SLOTS, PW], f32, tag=f"x{i}_{c}", name=f"x{i}_{c}")
                  for c in range(C)] for i in range(5)]  # i = di+2
            KT = pool.tile([128, OC, SLOTS, PW], f32, tag="kt")
            ACC = [pool.tile([128, SLOTS, PW], f32, tag=f"acc{c}", name=f"acc{c}")
                   for c in range(C)]
            TMP = pool.tile([128, SLOTS, PW], f32, tag="tmp")
            # pentad-A rows at partitions 0..49, pentad-B rows at 64..93
            # (matmul operands need base partition in {0,32,64}; sharing one
            # tile halves the per-partition SBUF footprint)
            RAB = pool.tile([64 + 10 * NB, RHSF], f32, tag="rab")
            RHSA, RHSB = RAB[0:10 * NA], RAB[64:64 + 10 * NB]
            WAB = pool.tile([64 + 10 * NB, 3, n_layers, 25 * NA], f32, tag="wab")
            WA, WB = WAB[0:10 * NA], WAB[64:64 + 10 * NB, :, :, 0:25 * NB]
            SC = pool.tile([128, n_layers * C], f32, tag="sc")
            BI = pool.tile([128, n_layers * C], f32, tag="bi")

            # one-time init
            for i in range(5):
                for c in range(C):
                    nc.vector.memset(X[i][c][:], 0.0)
            nc.vector.memset(RAB[:], 0.0)
            # ones rows (bias): compute engines need 32-aligned partition
            # bases, so fill row 9 of each group via DMA from a ones tile
            ONES = pool.tile([32, PW], f32, tag="ones")
            nc.vector.memset(ONES[:], 1.0)
            for g in range(G):
                rhs, gl = (RHSA, g) if g < NA else (RHSB, g - NA)
                nc.sync.dma_start(
                    out=rhs[10 * gl + 9:10 * gl + 10, 4:4 + GF], in_=ONES[:])
            nc.sync.dma_start(out=WA[:], in_=wa_in.rearrange("d l k m -> k d l m"))
            nc.sync.dma_start(out=WB[:], in_=wb_in.rearrange("d l k m -> k d l m"))
            nc.sync.dma_start(out=SC[:], in_=sc_in[:])
            nc.sync.dma_start(out=BI[:], in_=bi_in[:])
            for c in range(C):
                nc.sync.dma_start(
                    out=X[2][c][:, :, 2:2 + W],
                    in_=x_in[c].rearrange("(s p) w -> p s w", p=128),
                )

            def hshift_copies(c):
                # X[di+2][c][p, s, :] = plane row (p+128s)+di of X[2][c]
                for di in (-2, -1, 1, 2):
                    dst, src = X[di + 2][c], X[2][c]
                    r0, r1 = max(0, -di), H - max(0, di)
                    r = r0
                    while r < r1:
                        s, sp = r // 128, (r + di) // 128
                        seg = min(r1, (s + 1) * 128, (sp + 1) * 128 - di)
                        nc.sync.dma_start(
                            out=dst[r % 128:r % 128 + seg - r, s, :],
                            in_=src[(r + di) % 128:(r + di) % 128 + seg - r, sp, :],
                        )
                        r = seg

            def build_rhs():
                # rhs row (10g+q) <- group-g 32-row span of h-shifted plane q
                for di_i, di in enumerate((-1, 0, 1)):
                    for ic in range(C):
                        q = di_i * 3 + ic
                        for g in range(G):
                            rhs, gl = (RHSA, g) if g < NA else (RHSB, g - NA)
                            row = 10 * gl + q
                            nc.sync.dma_start(
                                out=rhs[row:row + 1, 4:4 + GF],
                                in_=X[di + 2][ic][32 * (g % 4):32 * (g % 4) + GR,
                                                  g // 4, :])

            KTg = KT.rearrange("(g pp) o s w -> g pp o s w", pp=GR)

            def conv(l):
                for t in range(GR // 2):  # pairs of within-group rows
                    # psum row stride padded to 512 (PSUM bank = 512 fp32;
                    # a matmul output cannot cross banks)
                    pa = pa_pool.tile([25 * NA, 2, 512], f32, tag="pa")
                    pb = pb_pool.tile([25 * NB, 2, 512], f32, tag="pb")
                    for dj_i, dj in enumerate((-1, 0, 1)):
                        lta = WA[:, dj_i, l, :]
                        ltb = WB[:, dj_i, l, :]
                        for rr in range(2):
                            f0 = 4 + (2 * t + rr) * PW + dj
                            nc.tensor.matmul(
                                pa[:, rr, 0:PW], lta, RHSA[:, f0:f0 + PW],
                                start=(dj_i == 0), stop=(dj_i == 2))
                        for rr in range(2):
                            f0 = 4 + (2 * t + rr) * PW + dj
                            nc.tensor.matmul(
                                pb[:, rr, 0:PW], ltb, RHSB[:, f0:f0 + PW],
                                start=(dj_i == 0), stop=(dj_i == 2))
                    # PSUM -> SBUF staging (DMA cannot read PSUM); lane-locked
                    ksa = ks_pool.tile([25 * NA, 2, PW], f32, tag="ksa")
                    ksb = ks_pool.tile([25 * NB, 2, PW], f32, tag="ksb")
                    nc.scalar.copy(ksa[:], pa[:, :, 0:PW])
                    nc.scalar.copy(ksb[:], pb[:, :, 0:PW])
                    # relayout 2 rows x 8 groups into pixel-partition KT:
                    # group g row (32g+2t+rr) -> partition (..)%128, slot //128
                    for g in range(G):
                        ks, gl = (ksa, g) if g < NA else (ksb, g - NA)
                        for rr in range(2):
                            r = 32 * g + 2 * t + rr
                            p, s = r % 128, r // 128
                            # ACT HWDGE ring: keeps evac descriptors off the
                            # SP ring that carries the build/hshift DMAs
                            nc.scalar.dma_start(
                                out=KT[p:p + 1, :, s, :],
                                in_=ks[25 * gl:25 * gl + 25, rr, :])

            def pixelconv_bn_relu(l):
                for c in range(C):
                    acc = ACC[c][:, :, 2:2 + W]
                    for i in range(5):          # di = i-2
                        for j in range(5):      # dj = j-2
                            p = i * 5 + j
                            xs = X[i][c][:, :, j:j + W]
                            kp = KT[:, p, :, 2:2 + W]
                            if p == 0:
                                nc.vector.tensor_mul(acc, xs, kp)
                            else:
                                nc.vector.tensor_mul(TMP[:, :, 2:2 + W], xs, kp)
                                nc.vector.tensor_add(acc, acc, TMP[:, :, 2:2 + W])
                    idx = l * C + c
                    nc.scalar.activation(
                        X[2][c][:, :, 2:2 + W], acc,
                        mybir.ActivationFunctionType.Relu,
                        bias=BI[:, idx:idx + 1], scale=SC[:, idx:idx + 1])

            for l in range(n_layers):
                # collapse cross-layer deps through one sync point: without
                # it, early-layer DMAs accumulate waits on many DMAHW
                # semaphore lanes and walrus rejects the DMA ("too many
                # sync wait commands")
                tc.strict_bb_all_engine_barrier()
                for c in range(C):
                    hshift_copies(c)
                build_rhs()
                conv(l)
                pixelconv_bn_relu(l)

            for c in range(C):
                nc.sync.dma_start(
                    out=y_out[c].rearrange("(s p) w -> p s w", p=128),
                    in_=X[2][c][:, :, 2:2 + W],
                )
    nc.compile()
    return nc


def get_program(n_layers=L):
    if n_layers not in _PROG_CACHE:
        _PROG_CACHE[n_layers] = _build_program(n_layers)
    return _PROG_CACHE[n_layers]


def prep_weights(Wk, bk, n_layers=L):
    """Host-side: blockdiag lhsT tensors wa [3,L,50,125], wb [3,L,30,75].

    out[25g+oc, n] = sum_q lhsT[10g+q, 25g+oc] * rhs[10g+q, n];
    q = di_i*3+ic rows hold Wk[oc, ic, di+1, dj+1], q=9 holds bias (dj=0 only).
    """
    Wk, bk = np.asarray(Wk, np.float32), np.asarray(bk, np.float32)
    wa = np.zeros((3, n_layers, 10 * NA, 25 * NA), np.float32)
    wb = np.zeros((3, n_layers, 10 * NB, 25 * NB), np.float32)
    for dj_i, dj in enumerate((-1, 0, 1)):
        for l in range(n_layers):
            for g in range(G):
                dst, gl = (wa, g) if g < NA else (wb, g - NA)
                for di_i, di in enumerate((-1, 0, 1)):
                    for ic in range(C):
                        q = di_i * 3 + ic
                        dst[dj_i, l, 10 * gl + q, 25 * gl:25 * gl + 25] = \
                            Wk[l, :, ic, di + 1, dj + 1]
                if dj == 0:
                    dst[dj_i, l, 10 * gl + 9, 25 * gl:25 * gl + 25] = bk[l]
    return wa, wb


def prep_bn(gamma, beta, run_mean, run_var, n_layers=L):
    gamma = np.asarray(gamma, np.float32)[:n_layers]
    beta = np.asarray(beta, np.float32)[:n_layers]
    mu = np.asarray(run_mean, np.float32)[:n_layers]
    var = np.asarray(run_var, np.float32)[:n_layers]
    sc = gamma / np.sqrt(var + EPS)
    bi = beta - mu * sc
    scb = np.ascontiguousarray(
        np.broadcast_to(sc.reshape(1, -1), (128, n_layers * C)), np.float32)
    bib = np.ascontiguousarray(
        np.broadcast_to(bi.reshape(1, -1), (128, n_layers * C)), np.float32)
    return scb, bib


def make_in_maps(x, Wk, bk, gamma, beta, run_mean, run_var, n_layers=L):
    x = np.asarray(x, np.float32)
    wa, wb = prep_weights(Wk, bk, n_layers)
    scb, bib = prep_bn(gamma, beta, run_mean, run_var, n_layers)
    return [
        {"x": np.ascontiguousarray(x[i]), "wa": wa, "wb": wb,
         "bnscale": scb, "bnbias": bib}
        for i in range(x.shape[0])
    ]


def kernel(x, Wk, bk, gamma, beta, run_mean, run_var):
    from concourse.bass_utils import run_bass_kernel_spmd

    nc = get_program(L)
    in_maps = make_in_maps(x, Wk, bk, gamma, beta, run_mean, run_var, L)
    res = run_bass_kernel_spmd(nc, in_maps, list(range(len(in_maps))))
    y = np.stack([r["y"] for r in res.results])
    return np.ascontiguousarray(y, np.float32)
